# revision 14
# baseline (speedup 1.0000x reference)
"""PoolNet (social-GAN pooling) Trainium2 kernel — fp8 DoubleRow edition.

Math (reference semantics, eval-mode BN):
  h1[f,i,j] = relu(bn1(concat(emb(pos_j - pos_i), h_j) @ W1 + b1))
  h2[f,i,j] = relu(bn2(h1 @ W2 + b2))
  out[f,i]  = max_j h2[f,i,j]

Algebraic reductions (as in the fp32 baseline):
  1. Layer 1 collapses: bn1(x@W1+b1) = u[f,j] - v[f,i] with
     u = pos@A' + h@W1h' + c1 and v = pos@A' (host-folded weights
     including the BN1 affine).  The (F,P,P,192) concat never exists.
  2. relu/bias are monotone, so max_j relu(z_j + c2) = relu(max_j z_j + c2).

fp8 path (2x PE throughput via MatmulPerfMode.DoubleRow):
  The moving tensor is CENTERED before quantization:
     X[i,j,k] = relu(u_jk - v_ik) - mu_jk,   mu = relu(u - vbar)
  which roughly halves the fp8 quantization error of both operands'
  contributions.  The exact correction  corr[j,d] = mu @ W2  is
  j-dependent, so it is accumulated into PSUM **by the PE itself** as a
  third DoubleRow matmul per tile: stationary = (C0, C1) = hi/lo fp8
  split of corr, moving = a static indicator tensor delta_{p=j(row)}
  carrying the split scales (192, 16).  Effective corr precision ~2^-8.

  Scales: u,v,mu pre-scaled by 64 at psum evacuation; W2q = fp8(128*W2');
  psum domain = 8192 * z;  final activation applies 1/8192 + c2 + relu.

Sharding: data-parallel over frames, 4 frames per core on 8 cores.
"""

import sys

for _p in ("/opt/trn_rl_repo",):
    if _p not in sys.path:
        sys.path.insert(0, _p)

from contextlib import ExitStack

import numpy as np

import concourse.bass as bass
import concourse.mybir as mybir
import concourse.tile as tile
from concourse import bacc
from concourse.bass_utils import run_bass_kernel_spmd
from concourse.masks import make_identity

EPS = 1e-5
F, P, B, H, E, M, D = 32, 64, 2048, 128, 64, 512, 1024
NCORES = 8
FC = F // NCORES  # frames per core
RPC = FC * P  # rows per core = 256
QK = M // 128  # layer-2 contraction chunks = 4
QM = D // 128  # layer-2 output chunks = 8

SH = 32.0  # u/v/mu pre-scale (moving-tensor fp8 scale)
SW = 128.0  # W2 fp8 scale; psum domain = SH*SW = 8192
A0, A1 = 192.0, 16.0  # corr hi/lo indicator scales (fp8-exact)

_CACHE = {}


def _build_nc(
    loop_iters=1,
    ih=8,
    t_bufs=6,
    pair_bufs=10,
    ps_bufs=4,
    ps_cols=512,
    evac_m=0,
    max2_pool_q=0,
    cast_act_q=4,
    cent_q=2,
    sub1_engine="gpsimd",
    ev_bufs=3,
    tail_spread=True,
):
    IH = ih
    HB = IH * P
    NBLK = RPC // IH
    BPF = P // IH  # blocks per frame
    f32 = mybir.dt.float32
    f32r = mybir.dt.float32r
    bf16 = mybir.dt.bfloat16
    fp8 = mybir.dt.float8e4
    AF = mybir.ActivationFunctionType
    ALU = mybir.AluOpType
    DR = mybir.MatmulPerfMode.DoubleRow

    nc = bacc.Bacc("TRN2", target_bir_lowering=False, debug=False)

    pos_t = nc.dram_tensor("pos_t", [2, RPC], f32r, kind="ExternalInput").ap()
    h_t = nc.dram_tensor("h_t", [H, RPC], f32r, kind="ExternalInput").ap()
    w2q_d = nc.dram_tensor("w2q", [128, QK, D], fp8, kind="ExternalInput").ap()
    w2c_d = nc.dram_tensor("w2c", [128, QK, D], bf16, kind="ExternalInput").ap()
    w1h = nc.dram_tensor("w1h", [H, M], f32r, kind="ExternalInput").ap()
    a2 = nc.dram_tensor("a2", [2, M], f32r, kind="ExternalInput").ap()
    c1c = nc.dram_tensor("c1c", [128, QK], f32, kind="ExternalInput").ap()
    c2c = nc.dram_tensor("c2c", [128, QM], f32, kind="ExternalInput").ap()
    ind_d = nc.dram_tensor("ind", [128, 2, HB], fp8, kind="ExternalInput").ap()
    ind2_d = nc.dram_tensor("ind2", [128, P // IH, HB], bf16, kind="ExternalInput").ap()
    zp_d = nc.dram_tensor("zp", [64, 2, D], fp8, kind="ExternalInput").ap()
    out = nc.dram_tensor("out", [RPC, D], f32, kind="ExternalOutput").ap()

    eng = {"vector": nc.vector, "gpsimd": nc.gpsimd}

    with ExitStack() as ctx:
        tc = ctx.enter_context(tile.TileContext(nc))
        consts = ctx.enter_context(tc.tile_pool(name="consts", bufs=1))
        data = ctx.enter_context(tc.tile_pool(name="data", bufs=1))

        w2q = consts.tile([128, QK, D], fp8)
        nc.sync.dma_start(out=w2q, in_=w2q_d)
        w2c = consts.tile([128, QK, D], bf16)
        nc.sync.dma_start(out=w2c, in_=w2c_d)
        w1hsb = consts.tile([H, M], f32r)
        nc.sync.dma_start(out=w1hsb, in_=w1h)
        a2sb = consts.tile([2, M], f32r)
        nc.sync.dma_start(out=a2sb, in_=a2)
        c1sb = consts.tile([128, QK], f32)
        nc.sync.dma_start(out=c1sb, in_=c1c)
        c2sb = consts.tile([128, QM], f32)
        nc.sync.dma_start(out=c2sb, in_=c2c)
        possb = consts.tile([2, RPC], f32r)
        nc.sync.dma_start(out=possb, in_=pos_t)
        htsb = consts.tile([H, RPC], f32r)
        nc.sync.dma_start(out=htsb, in_=h_t)
        ind = consts.tile([128, 2, HB], fp8)
        nc.sync.dma_start(out=ind, in_=ind_d)
        ind2 = consts.tile([128, P // IH, HB], bf16)
        nc.sync.dma_start(out=ind2, in_=ind2_d)
        ident = consts.tile([128, 128], f32)
        make_identity(nc, ident)

        # corr stationary tiles (double-buffered across frames); rows 64-127
        # stay zero forever
        lc0 = consts.tile([128, 2, D], fp8)
        lc1 = consts.tile([128, 2, D], fp8)
        lc2 = consts.tile([128, 2, D], fp8)
        lc3 = consts.tile([128, 2, D], fp8)
        lcs = [lc0, lc1, lc2, lc3]
        for _lc in lcs:
            nc.sync.dma_start(out=_lc[64:128], in_=zp_d)

        u_sb = data.tile([128, QK, RPC], f32)
        v_sb = data.tile([128, QK, RPC], f32)
        vn_sb = data.tile([128, QK, RPC], f32)
        mu_sb = data.tile([128, QK, RPC], f32)
        mu_bf = data.tile([128, QK, RPC], bf16)
        up_sb = data.tile([128, QK, RPC], f32)
        nmu_sb = data.tile([128, QK, RPC], bf16)
        nmu_exp = data.tile([128, QK, FC, HB], bf16)
        sp_sb = data.tile([128, QK, FC, 128], bf16)
        vbar_sb = data.tile([128, QK, FC], f32)
        pool_sb = data.tile([128, QM, RPC], bf16)
        out_sb = data.tile([128, 2, D], f32)

        tpool = ctx.enter_context(tc.tile_pool(name="tp", bufs=t_bufs))
        h1p = ctx.enter_context(tc.tile_pool(name="h1", bufs=pair_bufs))
        tmp = ctx.enter_context(tc.tile_pool(name="tmp", bufs=ev_bufs + 4))
        pspool = ctx.enter_context(tc.tile_pool(name="ps", bufs=ps_bufs, space="PSUM"))
        cpspool = ctx.enter_context(tc.tile_pool(name="cps", bufs=1, space="PSUM"))
        tps = ctx.enter_context(tc.tile_pool(name="tps", bufs=2, space="PSUM"))

        out_r = out.rearrange("(h p) c -> p h c", p=128)

        def body():
            # ---- u = 64*(pos@A' + h@W1h' + c1), v = 64*pos@A' (bf16) ----
            for q in range(QK):
                ms = slice(q * 128, (q + 1) * 128)
                psu = pspool.tile([128, ps_cols], f32, tag="ps")
                nc.tensor.matmul(
                    psu[:, :RPC], lhsT=w1hsb[:, ms], rhs=htsb, start=True, stop=False
                )
                nc.tensor.matmul(
                    psu[:, :RPC], lhsT=a2sb[:, ms], rhs=possb, start=False, stop=True
                )
                nc.scalar.activation(
                    u_sb[:, q], psu[:, :RPC], AF.Identity,
                    bias=c1sb[:, q : q + 1], scale=SH,
                )
                psv = pspool.tile([128, ps_cols], f32, tag="ps")
                nc.tensor.matmul(
                    psv[:, :RPC], lhsT=a2sb[:, ms], rhs=possb, start=True, stop=True
                )
                nc.scalar.mul(v_sb[:, q], psv[:, :RPC], SH)
                # -sum_i v over each frame -> vbar (scaled to -mean later)
                for f in range(FC):
                    fs = slice(f * P, (f + 1) * P)
                    nc.vector.tensor_reduce(
                        vbar_sb[:, q, f : f + 1], v_sb[:, q, fs],
                        axis=mybir.AxisListType.X, op=ALU.add, negate=True,
                    )
            # vbar := -mean_i v  (scale by 1/P), then mu = relu(u - vbar)
            vbarm = data.tile([128, QK, FC], f32, name="vbarm")
            nc.scalar.mul(
                vbarm.rearrange("p a b -> p (a b)"),
                vbar_sb.rearrange("p a b -> p (a b)"), 1.0 / P,
            )
            for q in range(QK):
                for f in range(FC):
                    fs = slice(f * P, (f + 1) * P)
                    nc.scalar.activation(
                        mu_sb[:, q, fs], u_sb[:, q, fs], AF.Relu,
                        bias=vbarm[:, q, f : f + 1], scale=1.0,
                    )
            nc.scalar.copy(
                mu_bf.rearrange("p a b -> p (a b)"),
                mu_sb.rearrange("p a b -> p (a b)"),
            )
            # u' = u - mu (centered chunks only), nmu = -mu, vn = -v
            nc.scalar.mul(
                vn_sb.rearrange("p a b -> p (a b)"),
                v_sb.rearrange("p a b -> p (a b)"), -1.0,
            )
            for q in range(QK):
                if q < cent_q:
                    nc.vector.tensor_sub(up_sb[:, q], u_sb[:, q], mu_sb[:, q])
                    nc.scalar.mul(nmu_sb[:, q], mu_sb[:, q], -1.0)
                else:
                    nc.scalar.copy(up_sb[:, q], u_sb[:, q])
            # expand nmu over the i-block dim via DMA (engines stay free)
            for q in range(cent_q):
                for f in range(FC):
                    nc.sync.dma_start(
                        out=nmu_exp[:, q, f].rearrange("p (a b) -> p a b", b=P),
                        in_=nmu_sb[:, q, f * P : (f + 1) * P]
                        .unsqueeze(1)
                        .broadcast_to((128, IH, P)),
                    )
            # stationary [u'^T ; -v^T] per (q, frame) via PE transposes
            for q in range(QK):
                for f in range(FC):
                    fs = slice(f * P, (f + 1) * P)
                    pt1 = cpspool.tile([64, 128], f32, tag="tr", name="pt1")
                    nc.tensor.transpose(pt1, up_sb[:, q, fs], ident)
                    nc.scalar.copy(sp_sb[0:64, q, f], pt1)
                    pt2 = cpspool.tile([64, 128], f32, tag="tr", name="pt2")
                    nc.tensor.transpose(pt2, vn_sb[:, q, fs], ident)
                    nc.scalar.copy(sp_sb[64:128, q, f], pt2)

            # ---- corr[j,d] = mu @ W2 (psum domain), hi/lo fp8 split ----
            def emit_corr(f):
                lc = lcs[f]
                fs = slice(f * P, (f + 1) * P)
                for h in range(2):
                    hs = slice(h * 512, (h + 1) * 512)
                    psc = cpspool.tile([64, 512], f32, tag="cps")
                    for q in range(cent_q):
                        nc.tensor.matmul(
                            psc, lhsT=mu_bf[:, q, fs], rhs=w2c[:, q, hs],
                            start=(q == 0), stop=(q == cent_q - 1),
                        )
                    nc.scalar.mul(lc[0:64, 0, hs], psc, 1.0 / A0)
                    d0 = tmp.tile([64, 512], f32, tag="d0")
                    nc.scalar.mul(d0, lc[0:64, 0, hs], A0)
                    rt = tmp.tile([64, 512], f32, tag="rt")
                    nc.vector.tensor_sub(rt, psc, d0)
                    nc.scalar.mul(lc[0:64, 1, hs], rt, 1.0 / A1)

            def emit_tail(half):
                for m in range(QM):
                    pb = tmp.tile([128, 128], f32, tag="pb")
                    nc.scalar.activation(
                        pb, pool_sb[:, m, half * 128 : (half + 1) * 128],
                        AF.Relu, bias=c2sb[:, m : m + 1], scale=1.0 / (SH * SW),
                    )
                    pst = pspool.tile([128, ps_cols], f32, tag="ps")
                    nc.tensor.transpose(pst[:, :128], pb, ident)
                    nc.scalar.copy(
                        out_sb[:, half, m * 128 : (m + 1) * 128], pst[:, :128]
                    )
                nc.sync.dma_start(out=out_r[:, half], in_=out_sb[:, half])

            # ---- main loop over i-blocks ----
            sub1 = eng[sub1_engine]
            NCH = ps_cols // 512  # matmul-group chunks per psum tile
            for f in range(FC):
                emit_corr(f)
            for blk in range(NBLK):
                i0 = blk * IH
                f = i0 // P
                lc = lcs[f]
                fs = slice(f * P, (f + 1) * P)
                pos = blk % BPF
                pairs = []
                for pi in range(QK // 2):
                    pair = h1p.tile([128, 2, HB], fp8, tag="pair", name="pair")
                    for sub in range(2):
                        q = pi * 2 + sub
                        pst_t = tps.tile([128, HB], f32, tag="pst", name="pst_t")
                        nc.tensor.matmul(
                            pst_t, lhsT=sp_sb[:, q, f], rhs=ind2[:, pos],
                            start=True, stop=True,
                        )
                        xv = pair[:, sub]
                        if q >= cent_q:
                            nc.scalar.activation(xv, pst_t, AF.Relu)
                        else:
                            nc.vector.tensor_tensor(
                                xv, pst_t, nmu_exp[:, q, f], op=ALU.max
                            )
                    pairs.append(pair)
                ips = ps_cols // P
                for m in range(QM):
                    ms = slice(m * 128, (m + 1) * 128)
                    for ip in range(HB // ps_cols):
                        ps = pspool.tile([128, ps_cols], f32, tag="ps")
                        for ch in range(NCH):
                            ns = slice(
                                ip * ps_cols + ch * 512, ip * ps_cols + ch * 512 + 512
                            )
                            cs = slice(ch * 512, ch * 512 + 512)
                            for pi in range(QK // 2):
                                nc.tensor.matmul(
                                    ps[:, cs], lhsT=w2q[:, 2 * pi : 2 * pi + 2, ms],
                                    rhs=pairs[pi][:, :, ns],
                                    start=(pi == 0), stop=False, perf_mode=DR,
                                )
                            nc.tensor.matmul(
                                ps[:, cs], lhsT=lc[:, :, ms], rhs=ind[:, :, ns],
                                start=False, stop=True, perf_mode=DR,
                            )
                        po = pool_sb[:, m, i0 + ip * ips : i0 + (ip + 1) * ips]
                        if m < evac_m:
                            ev = tmp.tile([128, ps_cols], bf16, tag="ev", name="ev")
                            nc.scalar.copy(ev, ps)
                            nc.vector.reduce_max(
                                po, ev.rearrange("p (a b) -> p a b", b=P),
                                axis=mybir.AxisListType.X,
                            )
                        else:
                            nc.vector.reduce_max(
                                po, ps.rearrange("p (a b) -> p a b", b=P),
                                axis=mybir.AxisListType.X,
                            )
                if tail_spread and (blk + 1) * IH % 128 == 0:
                    emit_tail(((blk + 1) * IH) // 128 - 1)
            if not tail_spread:
                emit_tail(0)
                emit_tail(1)

        if loop_iters == 1:
            body()
        else:
            with tc.For_i(0, loop_iters, 1):
                body()

    nc.compile()
    return nc


def _fold_weights(We, be, W1, b1, g1, beta1, W2, b2, g2, beta2, rm1, rv1, rm2, rv2):
    f8 = np.float64
    We, be, W1, b1 = We.astype(f8), be.astype(f8), W1.astype(f8), b1.astype(f8)
    g1, beta1, rm1, rv1 = (
        g1.astype(f8), beta1.astype(f8), rm1.astype(f8), rv1.astype(f8),
    )
    W2, b2, g2, beta2, rm2, rv2 = (
        W2.astype(f8), b2.astype(f8), g2.astype(f8),
        beta2.astype(f8), rm2.astype(f8), rv2.astype(f8),
    )
    s1 = g1 / np.sqrt(rv1 + EPS)
    W1e = W1[:E]
    Ap = (We @ W1e) * s1  # (2, M)
    W1hp = W1[E:] * s1  # (H, M)
    c1 = s1 * (be @ W1e + b1 - rm1) + beta1  # (M,)
    s2 = g2 / np.sqrt(rv2 + EPS)
    W2p = W2 * s2  # (M, D)
    c2 = s2 * (b2 - rm2) + beta2  # (D,)
    asf = lambda x: np.ascontiguousarray(x, dtype=np.float32)
    return (
        asf(Ap),
        asf(W1hp),
        asf((SH * c1).reshape(QK, 128).T),
        asf(W2p),
        asf(c2.reshape(QM, 128).T),
    )


def _make_ind(ih):
    import ml_dtypes

    HB = ih * P
    ind = np.zeros((128, 2, HB), np.float32)
    rows = np.arange(HB)
    ind[rows % P, 0, rows] = A0
    ind[rows % P, 1, rows] = A1
    return ind.astype(ml_dtypes.float8_e4m3)


def _make_ind2(ih):
    import ml_dtypes

    HB = ih * P
    npos = P // ih
    ind2 = np.zeros((128, npos, HB), np.float32)
    rows = np.arange(HB)
    j = rows % P
    il = rows // P
    for pos in range(npos):
        ind2[j, pos, rows] = 1.0
        ind2[64 + pos * ih + il, pos, rows] = 1.0
    return ind2.astype(ml_dtypes.bfloat16)


def _prepare_in_maps(curr_h_states, curr_pos, ih=8, **weights):
    import ml_dtypes

    e4 = ml_dtypes.float8_e4m3
    bf = ml_dtypes.bfloat16
    Ap, W1hp, c1c, W2p, c2c = _fold_weights(**weights)
    w2q = np.ascontiguousarray(
        (W2p * SW).astype(e4).reshape(QK, 128, D).transpose(1, 0, 2)
    )
    w2c = np.ascontiguousarray(
        (W2p * SW).astype(bf).reshape(QK, 128, D).transpose(1, 0, 2)
    )
    ind = _make_ind(ih)
    ind2 = _make_ind2(ih)
    zp = np.zeros((64, 2, D), e4)
    h_full = np.asarray(curr_h_states, dtype=np.float32).reshape(B, H)
    pos_full = np.asarray(curr_pos, dtype=np.float32)
    in_maps = []
    for c in range(NCORES):
        r0, r1 = c * RPC, (c + 1) * RPC
        in_maps.append(
            {
                "pos_t": np.ascontiguousarray(pos_full[r0:r1].T),
                "h_t": np.ascontiguousarray(h_full[r0:r1].T),
                "w2q": w2q,
                "w2c": w2c,
                "w1h": W1hp,
                "a2": Ap,
                "c1c": c1c,
                "c2c": c2c,
                "ind": ind,
                "ind2": ind2,
                "zp": zp,
            }
        )
    return in_maps


def _get_nc(loop_iters=1, **opts):
    key = ("nc", loop_iters, tuple(sorted(opts.items())))
    if key not in _CACHE:
        _CACHE[key] = _build_nc(loop_iters, **opts)
    return _CACHE[key]


def _make_in_maps(inputs, ih=8):
    return _prepare_in_maps(
        curr_h_states=inputs["curr_h_states"],
        curr_pos=inputs["curr_pos"],
        ih=ih,
        We=np.asarray(inputs["We"]),
        be=np.asarray(inputs["be"]),
        W1=np.asarray(inputs["W1"]),
        b1=np.asarray(inputs["b1"]),
        g1=np.asarray(inputs["g1"]),
        beta1=np.asarray(inputs["beta1"]),
        W2=np.asarray(inputs["W2"]),
        b2=np.asarray(inputs["b2"]),
        g2=np.asarray(inputs["g2"]),
        beta2=np.asarray(inputs["beta2"]),
        rm1=np.asarray(inputs["rm1"]),
        rv1=np.asarray(inputs["rv1"]),
        rm2=np.asarray(inputs["rm2"]),
        rv2=np.asarray(inputs["rv2"]),
    )


def run(inputs, trace=False, loop_iters=1, opts=None, **kw):
    """Build in_maps from full inputs, run on 8 cores, return BassKernelResults."""
    opts = opts or {}
    in_maps = _make_in_maps(inputs, ih=opts.get("ih", 8))
    nc = _get_nc(loop_iters, **opts)
    return run_bass_kernel_spmd(
        nc, in_maps, core_ids=list(range(NCORES)), trace=trace, **kw
    )


def kernel(**inputs):
    res = run(inputs, trace=False)
    return np.concatenate([res.results[c]["out"] for c in range(NCORES)], axis=0)


# revision 15
# speedup vs baseline: 1.5142x; 1.5142x over previous
"""PoolNet (social-GAN pooling) Trainium2 kernel — fp8 DoubleRow edition.

Math (reference semantics, eval-mode BN):
  h1[f,i,j] = relu(bn1(concat(emb(pos_j - pos_i), h_j) @ W1 + b1))
  h2[f,i,j] = relu(bn2(h1 @ W2 + b2))
  out[f,i]  = max_j h2[f,i,j]

Algebraic reductions (as in the fp32 baseline):
  1. Layer 1 collapses: bn1(x@W1+b1) = u[f,j] - v[f,i] with
     u = pos@A' + h@W1h' + c1 and v = pos@A' (host-folded weights
     including the BN1 affine).  The (F,P,P,192) concat never exists.
  2. relu/bias are monotone, so max_j relu(z_j + c2) = relu(max_j z_j + c2).

fp8 path (2x PE throughput via MatmulPerfMode.DoubleRow, measured):
  For cent_q of the 4 contraction chunks the moving tensor is CENTERED
  before quantization:
     X[i,j,k] = relu(u_jk - v_ik) - mu_jk,   mu = relu(u - vbar)
  which roughly halves those chunks' fp8 quantization error (both
  operands' contributions scale with |X|).  The exact correction
  corr[j,d] = mu @ W2 is j-dependent, so it is accumulated into PSUM
  **by the PE itself** as one extra DoubleRow matmul per tile:
  stationary = (C0, C1) = hi/lo fp8 split of corr, moving = a static
  indicator delta_{p=j(row)} carrying the split scales (192, 16) --
  effective corr precision ~2^-8.  The remaining chunks skip centering
  (error mixes in quadrature; cent_q=2 lands at ~1.55e-2 < 2e-2).

Engine assignment (HW-measured rates drove this):
  PE:  u/v matmuls; t' = [u'^T; -v^T] @ indicator (avoids the 5x DVE
       broadcast penalty); main fp8-DR matmuls + corr-DR; transposes.
  DVE: psum max-pool drain (~0.92 ns/elem, the irreducible bottleneck);
       fused max(t', -mu) -> fp8 for centered chunks.
  Act: relu(t') -> fp8 for uncentered chunks; psum evacuations; tails.
  DMA: nmu broadcast-expansion (sync engine, otherwise idle).

  Scales: u,v,mu pre-scaled by SH=32 at psum evacuation;
  W2q = fp8(SW*W2'), SW=128; psum domain = 4096*z; the final
  activation applies 1/(SH*SW) + c2 + relu.

Sharding: data-parallel over frames, 4 frames per core on 8 cores.
"""

import sys

for _p in ("/opt/trn_rl_repo",):
    if _p not in sys.path:
        sys.path.insert(0, _p)

from contextlib import ExitStack

import numpy as np

import concourse.bass as bass
import concourse.mybir as mybir
import concourse.tile as tile
from concourse import bacc
from concourse.bass_utils import run_bass_kernel_spmd
from concourse.masks import make_identity

EPS = 1e-5
F, P, B, H, E, M, D = 32, 64, 2048, 128, 64, 512, 1024
NCORES = 8
FC = F // NCORES  # frames per core
RPC = FC * P  # rows per core = 256
QK = M // 128  # layer-2 contraction chunks = 4
QM = D // 128  # layer-2 output chunks = 8

SH = 32.0  # u/v/mu pre-scale (moving-tensor fp8 scale)
SW = 128.0  # W2 fp8 scale; psum domain = SH*SW = 8192
A0, A1 = 192.0, 16.0  # corr hi/lo indicator scales (fp8-exact)

_CACHE = {}


def _build_nc(
    loop_iters=1,
    ih=8,
    t_bufs=6,
    pair_bufs=10,
    ps_bufs=4,
    ps_cols=512,
    evac_m=0,
    max2_pool_q=0,
    cast_act_q=4,
    cent_q=2,
    sub1_engine="gpsimd",
    ev_bufs=3,
    tail_spread=True,
):
    IH = ih
    HB = IH * P
    NBLK = RPC // IH
    BPF = P // IH  # blocks per frame
    f32 = mybir.dt.float32
    f32r = mybir.dt.float32r
    bf16 = mybir.dt.bfloat16
    fp8 = mybir.dt.float8e4
    AF = mybir.ActivationFunctionType
    ALU = mybir.AluOpType
    DR = mybir.MatmulPerfMode.DoubleRow

    nc = bacc.Bacc("TRN2", target_bir_lowering=False, debug=False)

    pos_t = nc.dram_tensor("pos_t", [2, RPC], f32r, kind="ExternalInput").ap()
    h_t = nc.dram_tensor("h_t", [H, RPC], f32r, kind="ExternalInput").ap()
    w2q_d = nc.dram_tensor("w2q", [128, QK, D], fp8, kind="ExternalInput").ap()
    w2c_d = nc.dram_tensor("w2c", [128, QK, D], bf16, kind="ExternalInput").ap()
    w1h = nc.dram_tensor("w1h", [H, M], f32r, kind="ExternalInput").ap()
    a2 = nc.dram_tensor("a2", [2, M], f32r, kind="ExternalInput").ap()
    c1c = nc.dram_tensor("c1c", [128, QK], f32, kind="ExternalInput").ap()
    c2c = nc.dram_tensor("c2c", [128, QM], f32, kind="ExternalInput").ap()
    ind_d = nc.dram_tensor("ind", [128, 2, HB], fp8, kind="ExternalInput").ap()
    ind2_d = nc.dram_tensor("ind2", [128, P // IH, HB], bf16, kind="ExternalInput").ap()
    zp_d = nc.dram_tensor("zp", [64, 2, D], fp8, kind="ExternalInput").ap()
    out = nc.dram_tensor("out", [RPC, D], f32, kind="ExternalOutput").ap()

    eng = {"vector": nc.vector, "gpsimd": nc.gpsimd}

    with ExitStack() as ctx:
        tc = ctx.enter_context(tile.TileContext(nc))
        consts = ctx.enter_context(tc.tile_pool(name="consts", bufs=1))
        data = ctx.enter_context(tc.tile_pool(name="data", bufs=1))

        w2q = consts.tile([128, QK, D], fp8)
        nc.sync.dma_start(out=w2q, in_=w2q_d)
        w2c = consts.tile([128, QK, D], bf16)
        nc.sync.dma_start(out=w2c, in_=w2c_d)
        w1hsb = consts.tile([H, M], f32r)
        nc.sync.dma_start(out=w1hsb, in_=w1h)
        a2sb = consts.tile([2, M], f32r)
        nc.sync.dma_start(out=a2sb, in_=a2)
        c1sb = consts.tile([128, QK], f32)
        nc.sync.dma_start(out=c1sb, in_=c1c)
        c2sb = consts.tile([128, QM], f32)
        nc.sync.dma_start(out=c2sb, in_=c2c)
        possb = consts.tile([2, RPC], f32r)
        nc.sync.dma_start(out=possb, in_=pos_t)
        htsb = consts.tile([H, RPC], f32r)
        nc.sync.dma_start(out=htsb, in_=h_t)
        ind = consts.tile([128, 2, HB], fp8)
        nc.sync.dma_start(out=ind, in_=ind_d)
        ind2 = consts.tile([128, P // IH, HB], bf16)
        nc.sync.dma_start(out=ind2, in_=ind2_d)
        ident = consts.tile([128, 128], f32)
        make_identity(nc, ident)

        # corr stationary tiles (double-buffered across frames); rows 64-127
        # stay zero forever
        lc0 = consts.tile([128, 2, D], fp8)
        lc1 = consts.tile([128, 2, D], fp8)
        lc2 = consts.tile([128, 2, D], fp8)
        lc3 = consts.tile([128, 2, D], fp8)
        lcs = [lc0, lc1, lc2, lc3]
        for _lc in lcs:
            nc.sync.dma_start(out=_lc[64:128], in_=zp_d)

        u_sb = data.tile([128, QK, RPC], f32)
        v_sb = data.tile([128, QK, RPC], f32)
        vn_sb = data.tile([128, QK, RPC], f32)
        mu_sb = data.tile([128, QK, RPC], f32)
        mu_bf = data.tile([128, QK, RPC], bf16)
        up_sb = data.tile([128, QK, RPC], f32)
        nmu_sb = data.tile([128, QK, RPC], bf16)
        nmu_exp = data.tile([128, QK, FC, HB], bf16)
        sp_sb = data.tile([128, QK, FC, 128], bf16)
        vbar_sb = data.tile([128, QK, FC], f32)
        pool_sb = data.tile([128, QM, RPC], bf16)
        out_sb = data.tile([128, 2, D], f32)

        tpool = ctx.enter_context(tc.tile_pool(name="tp", bufs=t_bufs))
        h1p = ctx.enter_context(tc.tile_pool(name="h1", bufs=pair_bufs))
        tmp = ctx.enter_context(tc.tile_pool(name="tmp", bufs=ev_bufs + 4))
        pspool = ctx.enter_context(tc.tile_pool(name="ps", bufs=ps_bufs, space="PSUM"))
        cpspool = ctx.enter_context(tc.tile_pool(name="cps", bufs=1, space="PSUM"))
        tps = ctx.enter_context(tc.tile_pool(name="tps", bufs=2, space="PSUM"))

        out_r = out.rearrange("(h p) c -> p h c", p=128)

        def body():
            # ---- u = 64*(pos@A' + h@W1h' + c1), v = 64*pos@A' (bf16) ----
            for q in range(QK):
                ms = slice(q * 128, (q + 1) * 128)
                psu = pspool.tile([128, ps_cols], f32, tag="ps")
                nc.tensor.matmul(
                    psu[:, :RPC], lhsT=w1hsb[:, ms], rhs=htsb, start=True, stop=False
                )
                nc.tensor.matmul(
                    psu[:, :RPC], lhsT=a2sb[:, ms], rhs=possb, start=False, stop=True
                )
                nc.scalar.activation(
                    u_sb[:, q], psu[:, :RPC], AF.Identity,
                    bias=c1sb[:, q : q + 1], scale=SH,
                )
                psv = pspool.tile([128, ps_cols], f32, tag="ps")
                nc.tensor.matmul(
                    psv[:, :RPC], lhsT=a2sb[:, ms], rhs=possb, start=True, stop=True
                )
                nc.scalar.mul(v_sb[:, q], psv[:, :RPC], SH)
                # -sum_i v over each frame -> vbar (scaled to -mean later)
                for f in range(FC):
                    fs = slice(f * P, (f + 1) * P)
                    nc.vector.tensor_reduce(
                        vbar_sb[:, q, f : f + 1], v_sb[:, q, fs],
                        axis=mybir.AxisListType.X, op=ALU.add, negate=True,
                    )
            # vbar := -mean_i v  (scale by 1/P), then mu = relu(u - vbar)
            vbarm = data.tile([128, QK, FC], f32, name="vbarm")
            nc.scalar.mul(
                vbarm.rearrange("p a b -> p (a b)"),
                vbar_sb.rearrange("p a b -> p (a b)"), 1.0 / P,
            )
            for q in range(QK):
                for f in range(FC):
                    fs = slice(f * P, (f + 1) * P)
                    nc.scalar.activation(
                        mu_sb[:, q, fs], u_sb[:, q, fs], AF.Relu,
                        bias=vbarm[:, q, f : f + 1], scale=1.0,
                    )
            nc.scalar.copy(
                mu_bf.rearrange("p a b -> p (a b)"),
                mu_sb.rearrange("p a b -> p (a b)"),
            )
            # u' = u - mu (centered chunks only), nmu = -mu, vn = -v
            nc.scalar.mul(
                vn_sb.rearrange("p a b -> p (a b)"),
                v_sb.rearrange("p a b -> p (a b)"), -1.0,
            )
            for q in range(QK):
                if q < cent_q:
                    nc.vector.tensor_sub(up_sb[:, q], u_sb[:, q], mu_sb[:, q])
                    nc.scalar.mul(nmu_sb[:, q], mu_sb[:, q], -1.0)
                else:
                    nc.scalar.copy(up_sb[:, q], u_sb[:, q])
            # expand nmu over the i-block dim via DMA (engines stay free)
            for q in range(cent_q):
                for f in range(FC):
                    nc.sync.dma_start(
                        out=nmu_exp[:, q, f].rearrange("p (a b) -> p a b", b=P),
                        in_=nmu_sb[:, q, f * P : (f + 1) * P]
                        .unsqueeze(1)
                        .broadcast_to((128, IH, P)),
                    )
            # stationary [u'^T ; -v^T] per (q, frame) via PE transposes
            for q in range(QK):
                for f in range(FC):
                    fs = slice(f * P, (f + 1) * P)
                    pt1 = cpspool.tile([64, 128], f32, tag="tr", name="pt1")
                    nc.tensor.transpose(pt1, up_sb[:, q, fs], ident)
                    nc.scalar.copy(sp_sb[0:64, q, f], pt1)
                    pt2 = cpspool.tile([64, 128], f32, tag="tr", name="pt2")
                    nc.tensor.transpose(pt2, vn_sb[:, q, fs], ident)
                    nc.scalar.copy(sp_sb[64:128, q, f], pt2)

            # ---- corr[j,d] = mu @ W2 (psum domain), hi/lo fp8 split ----
            def emit_corr(f):
                lc = lcs[f]
                fs = slice(f * P, (f + 1) * P)
                for h in range(2):
                    hs = slice(h * 512, (h + 1) * 512)
                    psc = cpspool.tile([64, 512], f32, tag="cps")
                    for q in range(cent_q):
                        nc.tensor.matmul(
                            psc, lhsT=mu_bf[:, q, fs], rhs=w2c[:, q, hs],
                            start=(q == 0), stop=(q == cent_q - 1),
                        )
                    nc.scalar.mul(lc[0:64, 0, hs], psc, 1.0 / A0)
                    d0 = tmp.tile([64, 512], f32, tag="d0")
                    nc.scalar.mul(d0, lc[0:64, 0, hs], A0)
                    rt = tmp.tile([64, 512], f32, tag="rt")
                    nc.vector.tensor_sub(rt, psc, d0)
                    nc.scalar.mul(lc[0:64, 1, hs], rt, 1.0 / A1)

            def emit_tail(half):
                for m in range(QM):
                    pb = tmp.tile([128, 128], f32, tag="pb")
                    nc.scalar.activation(
                        pb, pool_sb[:, m, half * 128 : (half + 1) * 128],
                        AF.Relu, bias=c2sb[:, m : m + 1], scale=1.0 / (SH * SW),
                    )
                    pst = pspool.tile([128, ps_cols], f32, tag="ps")
                    nc.tensor.transpose(pst[:, :128], pb, ident)
                    nc.scalar.copy(
                        out_sb[:, half, m * 128 : (m + 1) * 128], pst[:, :128]
                    )
                nc.sync.dma_start(out=out_r[:, half], in_=out_sb[:, half])

            # ---- main loop over i-blocks ----
            sub1 = eng[sub1_engine]
            NCH = ps_cols // 512  # matmul-group chunks per psum tile
            for f in range(FC):
                emit_corr(f)
            for blk in range(NBLK):
                i0 = blk * IH
                f = i0 // P
                lc = lcs[f]
                fs = slice(f * P, (f + 1) * P)
                pos = blk % BPF
                pairs = []
                for pi in range(QK // 2):
                    pair = h1p.tile([128, 2, HB], fp8, tag="pair", name="pair")
                    for sub in range(2):
                        q = pi * 2 + sub
                        pst_t = tps.tile([128, HB], f32, tag="pst", name="pst_t")
                        nc.tensor.matmul(
                            pst_t, lhsT=sp_sb[:, q, f], rhs=ind2[:, pos],
                            start=True, stop=True,
                        )
                        xv = pair[:, sub]
                        if q >= cent_q:
                            nc.scalar.activation(xv, pst_t, AF.Relu)
                        else:
                            nc.vector.tensor_tensor(
                                xv, pst_t, nmu_exp[:, q, f], op=ALU.max
                            )
                    pairs.append(pair)
                ips = ps_cols // P
                for m in range(QM):
                    ms = slice(m * 128, (m + 1) * 128)
                    for ip in range(HB // ps_cols):
                        ps = pspool.tile([128, ps_cols], f32, tag="ps")
                        for ch in range(NCH):
                            ns = slice(
                                ip * ps_cols + ch * 512, ip * ps_cols + ch * 512 + 512
                            )
                            cs = slice(ch * 512, ch * 512 + 512)
                            for pi in range(QK // 2):
                                nc.tensor.matmul(
                                    ps[:, cs], lhsT=w2q[:, 2 * pi : 2 * pi + 2, ms],
                                    rhs=pairs[pi][:, :, ns],
                                    start=(pi == 0), stop=False, perf_mode=DR,
                                )
                            nc.tensor.matmul(
                                ps[:, cs], lhsT=lc[:, :, ms], rhs=ind[:, :, ns],
                                start=False, stop=True, perf_mode=DR,
                            )
                        po = pool_sb[:, m, i0 + ip * ips : i0 + (ip + 1) * ips]
                        if m < evac_m:
                            ev = tmp.tile([128, ps_cols], bf16, tag="ev", name="ev")
                            nc.scalar.copy(ev, ps)
                            nc.vector.reduce_max(
                                po, ev.rearrange("p (a b) -> p a b", b=P),
                                axis=mybir.AxisListType.X,
                            )
                        else:
                            nc.vector.reduce_max(
                                po, ps.rearrange("p (a b) -> p a b", b=P),
                                axis=mybir.AxisListType.X,
                            )
                if tail_spread and (blk + 1) * IH % 128 == 0:
                    emit_tail(((blk + 1) * IH) // 128 - 1)
            if not tail_spread:
                emit_tail(0)
                emit_tail(1)

        if loop_iters == 1:
            body()
        else:
            with tc.For_i(0, loop_iters, 1):
                body()

    nc.compile()
    return nc


def _fold_weights(We, be, W1, b1, g1, beta1, W2, b2, g2, beta2, rm1, rv1, rm2, rv2):
    f8 = np.float64
    We, be, W1, b1 = We.astype(f8), be.astype(f8), W1.astype(f8), b1.astype(f8)
    g1, beta1, rm1, rv1 = (
        g1.astype(f8), beta1.astype(f8), rm1.astype(f8), rv1.astype(f8),
    )
    W2, b2, g2, beta2, rm2, rv2 = (
        W2.astype(f8), b2.astype(f8), g2.astype(f8),
        beta2.astype(f8), rm2.astype(f8), rv2.astype(f8),
    )
    s1 = g1 / np.sqrt(rv1 + EPS)
    W1e = W1[:E]
    Ap = (We @ W1e) * s1  # (2, M)
    W1hp = W1[E:] * s1  # (H, M)
    c1 = s1 * (be @ W1e + b1 - rm1) + beta1  # (M,)
    s2 = g2 / np.sqrt(rv2 + EPS)
    W2p = W2 * s2  # (M, D)
    c2 = s2 * (b2 - rm2) + beta2  # (D,)
    asf = lambda x: np.ascontiguousarray(x, dtype=np.float32)
    return (
        asf(Ap),
        asf(W1hp),
        asf((SH * c1).reshape(QK, 128).T),
        asf(W2p),
        asf(c2.reshape(QM, 128).T),
    )


def _make_ind(ih):
    import ml_dtypes

    HB = ih * P
    ind = np.zeros((128, 2, HB), np.float32)
    rows = np.arange(HB)
    ind[rows % P, 0, rows] = A0
    ind[rows % P, 1, rows] = A1
    return ind.astype(ml_dtypes.float8_e4m3)


def _make_ind2(ih):
    import ml_dtypes

    HB = ih * P
    npos = P // ih
    ind2 = np.zeros((128, npos, HB), np.float32)
    rows = np.arange(HB)
    j = rows % P
    il = rows // P
    for pos in range(npos):
        ind2[j, pos, rows] = 1.0
        ind2[64 + pos * ih + il, pos, rows] = 1.0
    return ind2.astype(ml_dtypes.bfloat16)


def _prepare_in_maps(curr_h_states, curr_pos, ih=8, **weights):
    import ml_dtypes

    e4 = ml_dtypes.float8_e4m3
    bf = ml_dtypes.bfloat16
    Ap, W1hp, c1c, W2p, c2c = _fold_weights(**weights)
    w2q = np.ascontiguousarray(
        (W2p * SW).astype(e4).reshape(QK, 128, D).transpose(1, 0, 2)
    )
    w2c = np.ascontiguousarray(
        (W2p * SW).astype(bf).reshape(QK, 128, D).transpose(1, 0, 2)
    )
    ind = _make_ind(ih)
    ind2 = _make_ind2(ih)
    zp = np.zeros((64, 2, D), e4)
    h_full = np.asarray(curr_h_states, dtype=np.float32).reshape(B, H)
    pos_full = np.asarray(curr_pos, dtype=np.float32)
    in_maps = []
    for c in range(NCORES):
        r0, r1 = c * RPC, (c + 1) * RPC
        in_maps.append(
            {
                "pos_t": np.ascontiguousarray(pos_full[r0:r1].T),
                "h_t": np.ascontiguousarray(h_full[r0:r1].T),
                "w2q": w2q,
                "w2c": w2c,
                "w1h": W1hp,
                "a2": Ap,
                "c1c": c1c,
                "c2c": c2c,
                "ind": ind,
                "ind2": ind2,
                "zp": zp,
            }
        )
    return in_maps


def _get_nc(loop_iters=1, **opts):
    key = ("nc", loop_iters, tuple(sorted(opts.items())))
    if key not in _CACHE:
        _CACHE[key] = _build_nc(loop_iters, **opts)
    return _CACHE[key]


def _make_in_maps(inputs, ih=8):
    return _prepare_in_maps(
        curr_h_states=inputs["curr_h_states"],
        curr_pos=inputs["curr_pos"],
        ih=ih,
        We=np.asarray(inputs["We"]),
        be=np.asarray(inputs["be"]),
        W1=np.asarray(inputs["W1"]),
        b1=np.asarray(inputs["b1"]),
        g1=np.asarray(inputs["g1"]),
        beta1=np.asarray(inputs["beta1"]),
        W2=np.asarray(inputs["W2"]),
        b2=np.asarray(inputs["b2"]),
        g2=np.asarray(inputs["g2"]),
        beta2=np.asarray(inputs["beta2"]),
        rm1=np.asarray(inputs["rm1"]),
        rv1=np.asarray(inputs["rv1"]),
        rm2=np.asarray(inputs["rm2"]),
        rv2=np.asarray(inputs["rv2"]),
    )


def run(inputs, trace=False, loop_iters=1, opts=None, **kw):
    """Build in_maps from full inputs, run on 8 cores, return BassKernelResults."""
    opts = opts or {}
    in_maps = _make_in_maps(inputs, ih=opts.get("ih", 8))
    nc = _get_nc(loop_iters, **opts)
    return run_bass_kernel_spmd(
        nc, in_maps, core_ids=list(range(NCORES)), trace=trace, **kw
    )


def kernel(**inputs):
    res = run(inputs, trace=False)
    return np.concatenate([res.results[c]["out"] for c in range(NCORES)], axis=0)
